# revision 21
# baseline (speedup 1.0000x reference)
"""Trainium2 Bass kernel for nn_Att_2_layer1 (ragged attention over boxes).

Computation (reference):
  v_proj = relu(v @ Wv.T + bv)            [N,K,H]
  q_proj = relu(q @ Wq.T + bq)            [N,H]
  joint  = v_proj * q_proj[:,None,:]      [N,K,H]
  logits = joint @ Wl[0] + bl             [N,K]
  pad_sequence(tags_attention) gather -> [B,S,T,K]   (identity when tags==1)
  w = masked_softmax(logits_batch, box_mask)

Sharding: data-parallel over the flat tag dim NB (8 cores x 1024 rows),
weights replicated.  Host pre-transposes v and q to [d, nk] bf16 layout
(zero on-device transposes).  Column order within a 128-n group:
j = q4*1152 + k*32 + m  (q4 = n//32 stripe, m = n%32), so the G-matmul
diag extract reduces contiguously.

Scheduling (v2):
  - Startup: the scalar HWDGE ring's FIRST dma is pre-armed by the runtime
    ~5us before engine streams boot, so it carries a combo blob
    [v group0 cols 0:2048 | wvt | wqt | small consts] -> first matmul ~6us
    earlier.  g0's remaining cols, qT and the mask/C tables follow on the
    scalar ring; groups 1-7 ride the sync ring; per-group out DMAs ride
    the sync ring too (sync engine is idle).
  - Steady state: per group, 9 x 512-col vproj chunks (2 dh-accumulated
    matmuls per hh half), relu+bias PSUM->SBUF copies split 11 Scalar /
    7 Vector; PSUM rotation bufs=3 per hh.
  - G phase of group g-1 is spread per-blk inside group g's chunk loop
    (blk0/1/2 after chunks 3/5/7): 4x32-row stripes packed via
    tile_position co-issue in the PE array, diag-extract mult on Vector,
    contiguous segment reduce on GpSimd.
  - Masked softmax is folded: z' = z*msl + C with C = bl*msl - 30*(1-msl)
    (host table), e2 = exp(z') via Scalar activation whose accum_out
    gives the denominator for free; masked boxes contribute exp(-30).
    The reference's +1e-13*sall term is ~1e-13 of the denominator.
"""

import os
import numpy as np

B, S, T, K = 128, 4, 16, 36
VD, QD, H = 256, 256, 256
NB = B * S * T              # 8192
NCORES = 8
NPC = NB // NCORES          # 1024 n-rows per core
SBN = 32                    # n-rows per superblock (stripe)
SBK = SBN * K               # 1152 nk per superblock
NG = 8                      # groups of 128 n per core
GK = 128 * K                # 4608 nk per group
FB = 384                    # free-dim block (3 per superblock, 12 k each)
VC = 512                    # vproj chunk width (one PSUM bank)
NVC = GK // VC              # 9 vproj chunks per group
C0W = 1024                  # combo blob carries group0 cols [0, C0W)
NC0 = C0W // VC             # chunks served by the combo blob

_CACHE = {}

# relu-copy engine per (chunk, hh): 'V' entries chosen so Vector gets 6
# of 18 copies (it also runs the diag mults + segment reduces).
_COPY_ENGINE = {}
for _c in range(NVC):
    for _hh in range(2):
        _COPY_ENGINE[(_c, _hh)] = "S"
for _key in [(0, 1), (2, 0), (3, 1), (5, 0), (6, 1), (8, 0)]:
    _COPY_ENGINE[_key] = "V"


def _build_module():
    import concourse.bass as bass
    import concourse.mybir as mybir
    import concourse.tile as tile
    from concourse import bacc
    from contextlib import ExitStack

    f32 = mybir.dt.float32
    bf16 = mybir.dt.bfloat16

    nc = bacc.Bacc("TRN2", target_bir_lowering=False)

    # combo: [vt g0 cols 0:C0W (dh0|dh1) | wvt 512 | wqt 512 | smalls 16]
    CW = 2 * C0W + 512 + 512 + 16
    combo_d = nc.dram_tensor("combo", [128, CW], bf16, kind="ExternalInput")
    # rest of group 0, split in 3 so chunk matmuls unblock piecewise
    # (dependencies are tile-granular)
    G0SPLIT = [(C0W, 2560), (2560, 3584), (3584, GK)]
    g0r_d = nc.dram_tensor("g0r", [128, 2 * (GK - C0W)], bf16,
                           kind="ExternalInput")
    vt_d = nc.dram_tensor("vt", [(NG - 1) * 128, 2 * GK], bf16,
                          kind="ExternalInput")
    qt_d = nc.dram_tensor("qt", [128, 2 * NPC], bf16, kind="ExternalInput")
    cb16_d = nc.dram_tensor("cb16", [128, SBK + NG * K], bf16,
                            kind="ExternalInput")
    c32f_d = nc.dram_tensor("c32f", [128, NG * K + 2], f32,
                            kind="ExternalInput")
    out_d = nc.dram_tensor("out_w", [NPC, K], f32, kind="ExternalOutput")

    with tile.TileContext(nc) as tc, ExitStack() as ctx:
        singles = ctx.enter_context(tc.tile_pool(name="singles", bufs=1))

        # ---- sync ring carries everything vproj needs first (it is the
        # high-priority ring): combo = [g0 cols 0:1024 | wvt | wqt | smalls],
        # then the rest of g0, then groups 1-7 (emitted in the main loop).
        combo = singles.tile([128, CW], bf16)
        nc.sync.dma_start(out=combo, in_=combo_d[:])
        g0rs = []
        for lo, hi in G0SPLIT:
            g0rp = singles.tile([128, 2, hi - lo], bf16, name=f"g0r{lo}")
            nc.sync.dma_start(
                out=g0rp,
                in_=bass.AP(g0r_d, lo - C0W,
                            [[2 * (GK - C0W), 128], [GK - C0W, 2],
                             [1, hi - lo]]))
            g0rs.append(g0rp)
        # ---- scalar ring: inputs not needed until ~2 groups in; their
        # dma_starts are emitted mid-group-0 (below) so they don't steal
        # DMA bandwidth from group 0/1 v-data during the ramp ----
        qT = singles.tile([128, 2, NPC], bf16)
        cb16 = singles.tile([128, SBK + NG * K], bf16)
        c32f = singles.tile([128, NG * K + 2], f32)
        wl = c32f[:, NG * K:NG * K + 2]

        vt0a = combo[:, 0:2 * C0W].rearrange("p (dh j) -> p dh j", dh=2, j=C0W)
        co = 2 * C0W
        wvt = combo[:, co:co + 512].rearrange("p (dh h) -> p dh h", dh=2, h=H)
        wqt = combo[:, co + 512:co + 1024].rearrange(
            "p (dh h) -> p dh h", dh=2, h=H)
        sm16 = combo[:, co + 1024:co + 1024 + 8]
        mdiag = cb16[:, 0:SBK]
        msm = cb16[:, SBK:SBK + NG * K]

        # pre-load the Scalar activation table and the GpSimd tensor-op
        # library during the initial DMA wait (first GpSimd tensor ops
        # otherwise take ~7.5us each)
        warm = singles.tile([128, 8], f32)
        nc.gpsimd.memset(warm, 0.0)
        warm2 = singles.tile([128, 8], f32)
        nc.scalar.activation(out=warm2[:, 0:1], in_=warm[:, 0:1],
                             func=mybir.ActivationFunctionType.Relu)
        nc.gpsimd.tensor_scalar_mul(warm2[:, 1:2], warm[:, 1:2], 1.0)
        nc.gpsimd.tensor_mul(warm2[:, 2:3], warm[:, 2:3], warm[:, 3:4])
        nc.gpsimd.tensor_add(warm2[:, 3:4], warm[:, 4:5], warm[:, 5:6])

        # small consts in f32 (bias APs): bv0 bv1 bq0 bq1 wl0 wl1
        smf = singles.tile([128, 8], f32)
        nc.vector.tensor_scalar_add(smf, sm16, 0.0)
        bv = smf[:, 0:2]
        bq = smf[:, 2:4]

        gT = singles.tile([128, 2, NPC], bf16)     # q_proj.T * Wl  [h, n]

        # ---------------- pools --------------------------------------------
        vin_pool = ctx.enter_context(tc.tile_pool(name="vin", bufs=4))
        vp_pool = ctx.enter_context(tc.tile_pool(name="vp", bufs=2))
        d_pool = ctx.enter_context(tc.tile_pool(name="dsb", bufs=2))
        vp_ps = ctx.enter_context(tc.tile_pool(name="vp_ps", bufs=3, space="PSUM"))
        g_ps = ctx.enter_context(tc.tile_pool(name="g_ps", bufs=2, space="PSUM"))

        # ~3.4us of dummy matmuls during the combo DMA wait flips the PE
        # HAM clock gate to 2.4GHz before real matmuls start
        dml = singles.tile([128, 128], bf16)
        dmr = singles.tile([128, 512], bf16)
        nc.vector.memset(dml, 0.0)
        nc.vector.memset(dmr, 0.0)
        for i in range(8):
            dps = vp_ps.tile([128, 512], f32, name=f"warmmm{i}", tag="v0")
            nc.tensor.matmul(dps, dml, dmr, start=True, stop=True)

        def emit_q_phase():
            for hh in range(2):
                for blk in range(2):  # n blocks of 512
                    ps = vp_ps.tile([128, 512], f32, name=f"qmm{hh}{blk}",
                                    tag=f"v{hh}")
                    for dh in range(2):
                        nc.tensor.matmul(
                            ps,
                            wqt[:, dh, hh * 128:(hh + 1) * 128],
                            qT[:, dh, blk * 512:(blk + 1) * 512],
                            start=(dh == 0), stop=(dh == 1),
                        )
                    tmp = singles.tile([128, 512], f32, name=f"qrelu{hh}{blk}")
                    if blk == 0:
                        nc.scalar.activation(
                            out=tmp, in_=ps,
                            func=mybir.ActivationFunctionType.Relu,
                            bias=bq[:, hh:hh + 1], scale=1.0,
                        )
                    else:
                        nc.vector.tensor_scalar(
                            out=tmp, in0=ps,
                            scalar1=bq[:, hh:hh + 1], scalar2=0.0,
                            op0=mybir.AluOpType.add, op1=mybir.AluOpType.max,
                        )
                    nc.vector.tensor_scalar_mul(
                        gT[:, hh, blk * 512:(blk + 1) * 512],
                        tmp, wl[:, hh:hh + 1])

        def emit_chunk(g, vtile, c):
            # one 512-col vproj chunk: 2 hh halves, 2 dh-accumulated matmuls
            for hh in range(2):
                ps = vp_ps.tile([128, VC], f32, name=f"ps{g}_{c}_{hh}",
                                tag=f"v{hh}")
                for dh in range(2):
                    if g == 0:
                        if c < NC0:
                            src = vt0a[:, dh, c * VC:(c + 1) * VC]
                        else:
                            pi = next(i for i, (lo, hi) in enumerate(G0SPLIT)
                                      if lo <= c * VC < hi)
                            lo = G0SPLIT[pi][0]
                            src = g0rs[pi][:, dh,
                                           c * VC - lo:(c + 1) * VC - lo]
                    else:
                        src = vtile[:, dh, c * VC:(c + 1) * VC]
                    nc.tensor.matmul(
                        ps,
                        wvt[:, dh, hh * 128:(hh + 1) * 128],
                        src,
                        start=(dh == 0), stop=(dh == 1),
                    )
                dst = vps[g][:, hh, c * VC:(c + 1) * VC]
                if _COPY_ENGINE[(c, hh)] == "S":
                    nc.scalar.activation(
                        out=dst, in_=ps,
                        func=mybir.ActivationFunctionType.Relu,
                        bias=bv[:, hh:hh + 1], scale=1.0,
                    )
                else:
                    nc.vector.tensor_scalar(
                        out=dst, in0=ps,
                        scalar1=bv[:, hh:hh + 1], scalar2=0.0,
                        op0=mybir.AluOpType.add, op1=mybir.AluOpType.max,
                    )

        z36s = {}

        def emit_g_blk(g, blk):
            # G-matmul for one 384-col blk: 4 stripes of 32 n' packed via
            # tile_position (co-issued), then diag mult (V) + seg reduce (GpS)
            vp = vps[g]
            if blk == 0:
                z36s[g] = d_pool.tile([128, K], f32, name=f"z36_{g}", tag="z36")
            gt = g_ps.tile([128, FB], f32, name=f"gt{g}_{blk}", tag="gt")
            for hh in range(2):
                for q4 in range(4):
                    stripe = 32 * q4
                    nc.tensor.matmul(
                        gt[stripe:stripe + SBN, :],
                        gT[:, hh, g * 128 + stripe:g * 128 + stripe + SBN],
                        vp[:, hh, q4 * SBK + blk * FB:q4 * SBK + (blk + 1) * FB],
                        start=(hh == 0), stop=(hh == 1),
                        tile_position=(0, stripe),
                        skip_group_check=True,
                    )
            dsb = d_pool.tile([128, FB], f32, name=f"dsb{g}_{blk}", tag="dsb")
            nc.vector.tensor_mul(dsb, gt, mdiag[:, blk * FB:(blk + 1) * FB])
            nc.vector.tensor_reduce(
                out=z36s[g][:, blk * 12:(blk + 1) * 12],
                in_=dsb.rearrange("p (k m) -> p k m", k=12, m=SBN),
                axis=mybir.AxisListType.X,
                op=mybir.AluOpType.add,
            )

        def emit_softmax(g):
            # w = e2 / sum(e2), e2 = exp(z*msl + C); C = bl*msl - 30*(1-msl)
            z36 = z36s.pop(g)
            vps.pop(g)
            # small ops ride GpSimd (idle) except the last group, where
            # fewer cross-engine hops shorten the tail chain
            se = nc.vector if g == NG - 1 else nc.gpsimd
            zc = d_pool.tile([128, K], f32, name=f"zc_{g}", tag="zc")
            se.tensor_mul(zc, z36, msm[:, g * K:(g + 1) * K])
            se.tensor_add(zc, zc, c32f[:, g * K:(g + 1) * K])
            e2 = d_pool.tile([128, K], f32, name=f"e2_{g}", tag="e2")
            s2 = d_pool.tile([128, 1], f32, name=f"s2_{g}", tag="s2")
            nc.scalar.activation(out=e2, in_=zc,
                                 func=mybir.ActivationFunctionType.Exp,
                                 accum_out=s2)
            rec = d_pool.tile([128, 1], f32, name=f"rec_{g}", tag="rec")
            nc.vector.reciprocal(out=rec, in_=s2)
            wgt = d_pool.tile([128, K], f32, name=f"wg_{g}", tag="wgt")
            se.tensor_scalar_mul(wgt, e2, rec)
            nc.sync.dma_start(
                out=bass.AP(out_d, g * 128 * K, [[K, 128], [1, K]]),
                in_=wgt)

        vps = {}

        # ---------------- software-pipelined main loop ---------------------
        # Group g's G phase is spread per-blk inside group g+1's chunk loop
        # so its rhs (vp of g) is fully relu'd -> no PE stalls on G.
        for g in range(NG):
            vtile = None
            if g > 0:
                vtile = vin_pool.tile([128, 2, GK], bf16, name=f"vt{g}",
                                      tag="vt")
                nc.sync.dma_start(
                    out=vtile,
                    in_=bass.AP(vt_d, (g - 1) * 128 * 2 * GK,
                                [[2 * GK, 128], [GK, 2], [1, GK]]))
            vps[g] = vp_pool.tile([128, 2, GK], bf16, name=f"vp{g}", tag="vp")
            for c in range(NVC):
                emit_chunk(g, vtile, c)
                if g == 0:
                    if c == 3:
                        nc.scalar.dma_start(
                            out=qT,
                            in_=bass.AP(qt_d, 0,
                                        [[2 * NPC, 128], [NPC, 2], [1, NPC]]))
                    elif c == 6:
                        nc.scalar.dma_start(out=cb16, in_=cb16_d[:])
                        nc.scalar.dma_start(out=c32f, in_=c32f_d[:])
                if c == 2 and g == 1:
                    emit_q_phase()
                if g >= 1:
                    if c == 3:
                        emit_g_blk(g - 1, 0)
                    elif c == 5:
                        emit_g_blk(g - 1, 1)
                    elif c == 7:
                        emit_g_blk(g - 1, 2)
                        emit_softmax(g - 1)
                        if g == NG - 1:
                            emit_g_blk(NG - 1, 0)
        for blk in range(1, 3):
            emit_g_blk(NG - 1, blk)
        emit_softmax(NG - 1)

    nc.finalize()
    return nc


def _host_prep(v, q, box_mask, Wv, bv, Wq, bq, Wl, bl):
    import ml_dtypes
    bf16 = ml_dtypes.bfloat16

    # vT [c, g, p, dh, j] with j = q4*1152 + k*32 + m, d = dh*128 + p
    vt = v.reshape(NCORES, NG, 4, SBN, K, VD).astype(bf16)
    vt = vt.transpose(0, 1, 5, 2, 4, 3)          # [c, g, d, q4, k, m]
    vt = vt.reshape(NCORES, NG, 2, 128, GK)
    vt = np.ascontiguousarray(vt.transpose(0, 1, 3, 2, 4))  # [c, g, p, dh, j]

    qt = q.reshape(NCORES, NPC, QD).astype(bf16)
    qt = qt.transpose(0, 2, 1).reshape(NCORES, 2, 128, NPC)
    qt = np.ascontiguousarray(qt.transpose(0, 2, 1, 3))     # [c, p, dh, n]
    qt = qt.reshape(NCORES, 128, 2 * NPC)

    # wvt[p, dh, h] = Wv[h, dh*128+p]
    wvt = Wv.T.reshape(2, 128, H).transpose(1, 0, 2).reshape(128, 512)
    wqt = Wq.T.reshape(2, 128, H).transpose(1, 0, 2).reshape(128, 512)
    smalls = np.zeros((128, 16), dtype=np.float32)
    smalls[:, 0] = bv[:128]
    smalls[:, 1] = bv[128:]
    smalls[:, 2] = bq[:128]
    smalls[:, 3] = bq[128:]
    smalls[:, 4] = Wl[0, :128]
    smalls[:, 5] = Wl[0, 128:]
    # mdiag[p, k*32 + m] = 1 iff m == p % 32
    mdiag = np.zeros((128, SBK), dtype=np.float32)
    for p in range(128):
        mdiag[p, (p % SBN)::SBN] = 1.0

    in_maps = []
    for c in range(NCORES):
        n0 = c * NPC
        combo = np.concatenate(
            [vt[c, 0, :, :, :C0W].reshape(128, 2 * C0W),
             wvt.astype(bf16).astype(np.float32),
             wqt.astype(bf16).astype(np.float32),
             smalls], axis=1).astype(bf16)
        g0r = np.ascontiguousarray(
            vt[c, 0, :, :, C0W:]).reshape(128, 2 * (GK - C0W))
        vtg = np.ascontiguousarray(vt[c, 1:]).reshape((NG - 1) * 128, 2 * GK)
        # msm[p, g*K + k] = box_mask[b(n)] with global n = n0 + g*128 + p
        nloc = (np.arange(NG)[None, :] * 128 + np.arange(128)[:, None])
        bidx = (n0 + nloc) // (S * T)          # [128, NG]
        msm = box_mask[bidx].reshape(128, NG * K).astype(np.float32)
        cb16 = np.ascontiguousarray(
            np.concatenate([mdiag, msm], axis=1)).astype(bf16)
        wlcols = np.stack([Wl[0, :128], Wl[0, 128:]], axis=1)
        c32f = np.ascontiguousarray(np.concatenate(
            [msm * bl[0] - 30.0 * (1.0 - msm), wlcols],
            axis=1)).astype(np.float32)
        in_maps.append(dict(combo=combo, g0r=g0r, vt=vtg, qt=qt[c],
                            cb16=cb16, c32f=c32f))
    return in_maps


def _numpy_fallback(v, q, box_mask, tags_attention, Wv, bv, Wq, bq, Wl, bl):
    v_proj = np.maximum(v @ Wv.T + bv, 0.0)
    q_proj = np.maximum(q @ Wq.T + bq, 0.0)
    logits = (v_proj * q_proj[:, None, :]) @ Wl[0] + bl[0]
    lengths = tags_attention.sum(-1)
    flat_len = lengths.reshape(-1)
    offsets = np.concatenate([[0], np.cumsum(flat_len)[:-1]]).reshape(B, S)
    t = np.arange(T)
    idx = offsets[:, :, None] + t
    valid = t[None, None, :] < lengths[:, :, None]
    gathered = logits[np.clip(idx, 0, logits.shape[0] - 1)]
    lb = np.where(valid[..., None], gathered, 0.0)
    mask = box_mask[:, None, None, :]
    zz = lb * mask
    zz = zz - zz.max(-1, keepdims=True)
    ee = np.exp(zz)
    sm = ee / ee.sum(-1, keepdims=True)
    w = sm * mask
    w = w / (w.sum(-1, keepdims=True) + 1e-13)
    return w.astype(np.float32)


def kernel(v, q, box_mask, tags_attention, Wv, bv, Wq, bq, Wl, bl):
    v = np.asarray(v, dtype=np.float32)
    q = np.asarray(q, dtype=np.float32)
    box_mask = np.asarray(box_mask, dtype=np.float32)
    tags = np.asarray(tags_attention)
    Wv = np.asarray(Wv, dtype=np.float32); bv = np.asarray(bv, dtype=np.float32)
    Wq = np.asarray(Wq, dtype=np.float32); bq = np.asarray(bq, dtype=np.float32)
    Wl = np.asarray(Wl, dtype=np.float32); bl = np.asarray(bl, dtype=np.float32)

    if not np.all(tags == 1):
        return _numpy_fallback(v, q, box_mask, tags, Wv, bv, Wq, bq, Wl, bl)

    from concourse.bass_utils import run_bass_kernel_spmd

    if "nc" not in _CACHE:
        _CACHE["nc"] = _build_module()
    nc = _CACHE["nc"]

    in_maps = _host_prep(v, q, box_mask, Wv, bv, Wq, bq, Wl, bl)
    res = run_bass_kernel_spmd(
        nc, in_maps, core_ids=list(range(NCORES)),
        trace=bool(int(os.environ.get("BASS_KERNEL_TRACE", "0"))),
    )
    _CACHE["last_results"] = res
    w = np.concatenate([r["out_w"] for r in res.results], axis=0)
    return np.ascontiguousarray(w.reshape(B, S, T, K))
